# revision 4
# baseline (speedup 1.0000x reference)
"""Block-wise (128x128) min/max quantization observer kernel for TRN2.

Computes per-block scale / zero_point over an [8192, 8192] f32 tensor and
replicates each block's params over its 128x128 region, returning full-shape
scale (f32) and zero_point (i32) tensors — matching the jax reference
bit-exactly on the non-degenerate path.

Sharding: 8 NeuronCores, each handles a 1024-row stripe (8 row-blocks),
fully independent (embarrassingly parallel over row-blocks).
"""

import numpy as np

ROWS, COLS = 8192, 8192
BR, BC = 128, 128
N_CORES = 8
ROWS_PER_CORE = ROWS // N_CORES          # 1024
RB_PER_CORE = ROWS_PER_CORE // BR        # 8 row-blocks per core
COL_CHUNK = 4096                         # process half the columns at a time
N_CHUNKS = COLS // COL_CHUNK             # 2
NBLK = COL_CHUNK // BC                   # 32 col-blocks per chunk

# 1.5 * 2**23: adding/subtracting this in fp32 rounds to nearest-even integer
# for |x| < 2**22 (the round-half-even jnp.round behavior).
RNE_MAGIC = 12582912.0
# fp32(1/255), the multiplier neuron's fp32 divide-by-255 uses.
R255 = float(np.float32(1.0) / np.float32(255.0))

_CACHE = {}


def _build():
    import concourse.bacc as bacc
    import concourse.tile as tile
    import concourse.mybir as mybir
    from concourse import bass_isa

    f32 = mybir.dt.float32
    i32 = mybir.dt.int32
    Alu = mybir.AluOpType

    nc = bacc.Bacc(
        "TRN2",
        target_bir_lowering=False,
        debug=False,
        num_devices=N_CORES,
    )
    obs = nc.dram_tensor(
        "observed", [ROWS_PER_CORE, COLS], f32, kind="ExternalInput"
    ).ap()
    scale_dram = nc.dram_tensor(
        "scale", [ROWS_PER_CORE, COLS], f32, kind="ExternalOutput"
    ).ap()
    zp_dram = nc.dram_tensor(
        "zero_point", [ROWS_PER_CORE, COLS], i32, kind="ExternalOutput"
    ).ap()

    with tile.TileContext(nc) as tc:
        with (
            tc.tile_pool(name="pin", bufs=3) as pin,
            tc.tile_pool(name="pscale", bufs=3) as pscale,
            tc.tile_pool(name="pzp", bufs=3) as pzp,
            tc.tile_pool(name="psmall", bufs=4) as psmall,
        ):
            for rb in range(RB_PER_CORE):
                for h in range(N_CHUNKS):
                    r0 = rb * BR
                    c0 = h * COL_CHUNK
                    x = pin.tile([BR, COL_CHUNK], f32)
                    nc.sync.dma_start(
                        out=x, in_=obs[r0 : r0 + BR, c0 : c0 + COL_CHUNK]
                    )
                    x3 = x.rearrange("p (b c) -> p b c", c=BC)

                    # Free-dim (within-row) block partials: [128, NBLK]
                    pmax = psmall.tile([BR, NBLK], f32)
                    pmin = psmall.tile([BR, NBLK], f32)
                    nc.vector.tensor_reduce(
                        out=pmax, in_=x3, axis=mybir.AxisListType.X, op=Alu.max
                    )
                    nc.vector.tensor_reduce(
                        out=pmin, in_=x3, axis=mybir.AxisListType.X, op=Alu.min
                    )
                    negpmin = psmall.tile([BR, NBLK], f32)
                    nc.vector.tensor_scalar_mul(negpmin, pmin, -1.0)

                    # Cross-partition reduce (+ broadcast to all partitions).
                    bmax = psmall.tile([BR, NBLK], f32)
                    negbmin = psmall.tile([BR, NBLK], f32)
                    nc.gpsimd.partition_all_reduce(
                        bmax, pmax, channels=BR, reduce_op=bass_isa.ReduceOp.max
                    )
                    nc.gpsimd.partition_all_reduce(
                        negbmin, negpmin, channels=BR, reduce_op=bass_isa.ReduceOp.max
                    )

                    # Per-block qparams, replicated on every partition.
                    bmin = psmall.tile([BR, NBLK], f32)
                    nc.vector.tensor_scalar_mul(bmin, negbmin, -1.0)
                    rng = psmall.tile([BR, NBLK], f32)
                    nc.vector.tensor_tensor(rng, bmax, bmin, Alu.subtract)
                    deg = psmall.tile([BR, NBLK], f32)
                    nc.vector.tensor_scalar(deg, rng, 0.0, None, Alu.is_equal)
                    # scale = rng * (1/255) + deg  (== 1.0 when degenerate).
                    # Matches the on-device jax reference bit-exactly: neuron
                    # lowers fp32 divide to reciprocal+multiply.
                    scale_b = psmall.tile([BR, NBLK], f32)
                    nc.vector.scalar_tensor_tensor(
                        scale_b, rng, R255, deg, Alu.mult, Alu.add
                    )
                    # t = bmin / scale  ==  bmin * exact_reciprocal(scale)
                    rcp = psmall.tile([BR, NBLK], f32)
                    nc.vector.reciprocal(rcp, scale_b)
                    t = psmall.tile([BR, NBLK], f32)
                    nc.vector.tensor_tensor(t, bmin, rcp, Alu.mult)
                    # zpf = -t - 128 == qmin - bmin/scale
                    zpf = psmall.tile([BR, NBLK], f32)
                    nc.vector.tensor_scalar(
                        zpf, t, -1.0, -128.0, Alu.mult, Alu.add
                    )
                    # round to nearest even
                    zpr = psmall.tile([BR, NBLK], f32)
                    nc.vector.tensor_scalar(
                        zpr, zpf, RNE_MAGIC, RNE_MAGIC, Alu.add, Alu.subtract
                    )
                    notdeg = psmall.tile([BR, NBLK], f32)
                    nc.vector.tensor_scalar(
                        notdeg, deg, -1.0, 1.0, Alu.mult, Alu.add
                    )
                    # zp = zpr * (1 - deg), converted to int32 (integer-valued)
                    zp_b = psmall.tile([BR, NBLK], i32)
                    nc.vector.tensor_tensor(zp_b, zpr, notdeg, Alu.mult)

                    # Replicate each block value over its 128-column span.
                    scale_stripe = pscale.tile([BR, COL_CHUNK], f32)
                    nc.vector.tensor_copy(
                        out=scale_stripe.rearrange("p (b c) -> p b c", c=BC),
                        in_=scale_b.unsqueeze(2).broadcast_to([BR, NBLK, BC]),
                    )
                    zp_stripe = pzp.tile([BR, COL_CHUNK], i32)
                    nc.vector.tensor_copy(
                        out=zp_stripe.rearrange("p (b c) -> p b c", c=BC),
                        in_=zp_b.unsqueeze(2).broadcast_to([BR, NBLK, BC]),
                    )

                    nc.scalar.dma_start(
                        out=scale_dram[r0 : r0 + BR, c0 : c0 + COL_CHUNK],
                        in_=scale_stripe,
                    )
                    nc.scalar.dma_start(
                        out=zp_dram[r0 : r0 + BR, c0 : c0 + COL_CHUNK],
                        in_=zp_stripe,
                    )

    nc.compile()
    return nc


def _get_nc():
    if "nc" not in _CACHE:
        _CACHE["nc"] = _build()
    return _CACHE["nc"]


def _make_runner():
    """Jitted shard_map callable: full [8192,8192] in -> full-shape outs.

    Binds the bass_exec primitive directly (no zero-output donation — the
    kernel writes every output byte), sharding axis 0 across the 8 cores.
    """
    import jax
    import numpy as _np
    from jax.sharding import Mesh, PartitionSpec
    from jax.experimental.shard_map import shard_map
    from concourse import bass2jax
    import concourse.mybir as mybir

    nc = _get_nc()
    bass2jax.install_neuronx_cc_hook()

    partition_name = (
        nc.partition_id_tensor.name if nc.partition_id_tensor else None
    )
    in_names, out_names, out_avals = [], [], []
    for alloc in nc.m.functions[0].allocations:
        if not isinstance(alloc, mybir.MemoryLocationSet):
            continue
        name = alloc.memorylocations[0].name
        if alloc.kind == "ExternalInput":
            if name != partition_name:
                in_names.append(name)
        elif alloc.kind == "ExternalOutput":
            out_names.append(name)
            out_avals.append(
                jax.core.ShapedArray(
                    tuple(alloc.tensor_shape), mybir.dt.np(alloc.dtype)
                )
            )
    bind_in_names = list(in_names)
    if partition_name is not None:
        bind_in_names.append(partition_name)

    def _body(*args):
        operands = list(args)
        if partition_name is not None:
            operands.append(bass2jax.partition_id_tensor())
        outs = bass2jax._bass_exec_p.bind(
            *operands,
            out_avals=tuple(out_avals),
            in_names=tuple(bind_in_names),
            out_names=tuple(out_names),
            lowering_input_output_aliases=(),
            sim_require_finite=True,
            sim_require_nnan=True,
            nc=nc,
        )
        return tuple(outs)

    devices = jax.devices()[:N_CORES]
    assert len(devices) == N_CORES
    mesh = Mesh(_np.asarray(devices), ("core",))
    fn = jax.jit(
        shard_map(
            _body,
            mesh=mesh,
            in_specs=(PartitionSpec("core"),) * len(in_names),
            out_specs=(PartitionSpec("core"),) * len(out_names),
            check_rep=False,
        )
    )
    return fn, out_names, mesh


def _get_runner():
    if "runner" not in _CACHE:
        _CACHE["runner"] = _make_runner()
    return _CACHE["runner"]


def kernel(**inputs):
    observed = np.asarray(inputs["observed"], dtype=np.float32)
    assert observed.shape == (ROWS, COLS)
    fn, out_names, _ = _get_runner()
    outs = fn(observed)
    by_name = dict(zip(out_names, outs))
    return np.asarray(by_name["scale"]), np.asarray(by_name["zero_point"])


# revision 9
# speedup vs baseline: 429.7193x; 429.7193x over previous
"""Block-wise (128x128) min/max quantization observer kernel for TRN2.

Computes per-block scale / zero_point over an [8192, 8192] f32 tensor and
replicates each block's params over its 128x128 region, returning full-shape
scale (f32) and zero_point (i32) tensors — matching the jax reference
bit-exactly on the non-degenerate path.

Sharding: 8 NeuronCores, each handles a 1024-row stripe (8 row-blocks),
fully independent (embarrassingly parallel over row-blocks).
"""

import numpy as np

ROWS, COLS = 8192, 8192
BR, BC = 128, 128
N_CORES = 8
ROWS_PER_CORE = ROWS // N_CORES          # 1024
RB_PER_CORE = ROWS_PER_CORE // BR        # 8 row-blocks per core
COL_CHUNK = 4096                         # process half the columns at a time
N_CHUNKS = COLS // COL_CHUNK             # 2
NBLK = COL_CHUNK // BC                   # 32 col-blocks per chunk

# 1.5 * 2**23: adding/subtracting this in fp32 rounds to nearest-even integer
# for |x| < 2**22 (the round-half-even jnp.round behavior).
RNE_MAGIC = 12582912.0
# fp32(1/255), the multiplier neuron's fp32 divide-by-255 uses.
R255 = float(np.float32(1.0) / np.float32(255.0))

_CACHE = {}


def _build(reps=1):
    import concourse.bacc as bacc
    import concourse.tile as tile
    import concourse.mybir as mybir
    from concourse import bass_isa

    f32 = mybir.dt.float32
    i32 = mybir.dt.int32
    Alu = mybir.AluOpType

    nc = bacc.Bacc(
        "TRN2",
        target_bir_lowering=False,
        debug=False,
        num_devices=N_CORES,
    )
    obs = nc.dram_tensor(
        "observed", [ROWS_PER_CORE, COLS], f32, kind="ExternalInput"
    ).ap()
    scale_dram = nc.dram_tensor(
        "scale", [ROWS_PER_CORE, COLS], f32, kind="ExternalOutput"
    ).ap()
    zp_dram = nc.dram_tensor(
        "zero_point", [ROWS_PER_CORE, COLS], i32, kind="ExternalOutput"
    ).ap()

    with tile.TileContext(nc) as tc:
        with (
            tc.tile_pool(name="pin", bufs=3) as pin,
            tc.tile_pool(name="pscale", bufs=3) as pscale,
            tc.tile_pool(name="pzp", bufs=3) as pzp,
            tc.tile_pool(name="psmall", bufs=4) as psmall,
        ):

            def emit_unit(rb, h):
                r0 = rb * BR
                c0 = h * COL_CHUNK
                x = pin.tile([BR, COL_CHUNK], f32, name="x")
                nc.sync.dma_start(
                    out=x, in_=obs[r0 : r0 + BR, c0 : c0 + COL_CHUNK]
                )
                x3 = x.rearrange("p (b c) -> p b c", c=BC)

                # Free-dim (within-row) block partials: [128, NBLK]
                pmax = psmall.tile([BR, NBLK], f32, name="pmax")
                pmin = psmall.tile([BR, NBLK], f32, name="pmin")
                nc.vector.tensor_reduce(
                    out=pmax, in_=x3, axis=mybir.AxisListType.X, op=Alu.max
                )
                nc.vector.tensor_reduce(
                    out=pmin, in_=x3, axis=mybir.AxisListType.X, op=Alu.min
                )
                negpmin = psmall.tile([BR, NBLK], f32, name="negpmin")
                nc.vector.tensor_scalar_mul(negpmin, pmin, -1.0)

                # Cross-partition reduce (+ broadcast to all partitions).
                bmax = psmall.tile([BR, NBLK], f32, name="bmax")
                negbmin = psmall.tile([BR, NBLK], f32, name="negbmin")
                nc.gpsimd.partition_all_reduce(
                    bmax, pmax, channels=BR, reduce_op=bass_isa.ReduceOp.max
                )
                nc.gpsimd.partition_all_reduce(
                    negbmin, negpmin, channels=BR, reduce_op=bass_isa.ReduceOp.max
                )

                # Per-block qparams, replicated on every partition.
                bmin = psmall.tile([BR, NBLK], f32, name="bmin")
                nc.vector.tensor_scalar_mul(bmin, negbmin, -1.0)
                rng = psmall.tile([BR, NBLK], f32, name="rng")
                nc.vector.tensor_tensor(rng, bmax, bmin, Alu.subtract)
                deg = psmall.tile([BR, NBLK], f32, name="deg")
                nc.vector.tensor_scalar(deg, rng, 0.0, None, Alu.is_equal)
                # scale = rng * (1/255) + deg  (== 1.0 when degenerate).
                # Matches the on-device jax reference bit-exactly: neuron
                # lowers fp32 divide to reciprocal+multiply.
                scale_b = psmall.tile([BR, NBLK], f32, name="scale_b")
                nc.vector.scalar_tensor_tensor(
                    scale_b, rng, R255, deg, Alu.mult, Alu.add
                )
                # t = bmin / scale  ==  bmin * exact_reciprocal(scale)
                rcp = psmall.tile([BR, NBLK], f32, name="rcp")
                nc.vector.reciprocal(rcp, scale_b)
                t = psmall.tile([BR, NBLK], f32, name="t")
                nc.vector.tensor_tensor(t, bmin, rcp, Alu.mult)
                # zpf = -t - 128 == qmin - bmin/scale
                zpf = psmall.tile([BR, NBLK], f32, name="zpf")
                nc.vector.tensor_scalar(zpf, t, -1.0, -128.0, Alu.mult, Alu.add)
                # round to nearest even
                zpr = psmall.tile([BR, NBLK], f32, name="zpr")
                nc.vector.tensor_scalar(
                    zpr, zpf, RNE_MAGIC, RNE_MAGIC, Alu.add, Alu.subtract
                )
                notdeg = psmall.tile([BR, NBLK], f32, name="notdeg")
                nc.vector.tensor_scalar(notdeg, deg, -1.0, 1.0, Alu.mult, Alu.add)
                # zp = zpr * (1 - deg), converted to int32 (integer-valued)
                zp_b = psmall.tile([BR, NBLK], i32, name="zp_b")
                nc.vector.tensor_tensor(zp_b, zpr, notdeg, Alu.mult)

                # Replicate each block value over its 128-column span.
                scale_stripe = pscale.tile([BR, COL_CHUNK], f32, name="scale_stripe")
                nc.vector.tensor_copy(
                    out=scale_stripe.rearrange("p (b c) -> p b c", c=BC),
                    in_=scale_b.unsqueeze(2).broadcast_to([BR, NBLK, BC]),
                )
                zp_stripe = pzp.tile([BR, COL_CHUNK], i32, name="zp_stripe")
                nc.vector.tensor_copy(
                    out=zp_stripe.rearrange("p (b c) -> p b c", c=BC),
                    in_=zp_b.unsqueeze(2).broadcast_to([BR, NBLK, BC]),
                )

                nc.scalar.dma_start(
                    out=scale_dram[r0 : r0 + BR, c0 : c0 + COL_CHUNK],
                    in_=scale_stripe,
                )
                nc.scalar.dma_start(
                    out=zp_dram[r0 : r0 + BR, c0 : c0 + COL_CHUNK],
                    in_=zp_stripe,
                )

            for _rep in range(reps):
                for rb in range(RB_PER_CORE):
                    for h in range(N_CHUNKS):
                        emit_unit(rb, h)

    nc.compile()
    return nc


def _get_nc():
    if "nc" not in _CACHE:
        _CACHE["nc"] = _build()
    return _CACHE["nc"]


def _make_runner(nc=None):
    """Jitted shard_map callable: full [8192,8192] in -> full-shape outs.

    Binds the bass_exec primitive directly (no zero-output donation — the
    kernel writes every output byte), sharding axis 0 across the 8 cores.
    """
    import jax
    import numpy as _np
    from jax.sharding import Mesh, PartitionSpec
    from jax.experimental.shard_map import shard_map
    from concourse import bass2jax
    import concourse.mybir as mybir

    if nc is None:
        nc = _get_nc()
    bass2jax.install_neuronx_cc_hook()

    partition_name = (
        nc.partition_id_tensor.name if nc.partition_id_tensor else None
    )
    in_names, out_names, out_avals = [], [], []
    for alloc in nc.m.functions[0].allocations:
        if not isinstance(alloc, mybir.MemoryLocationSet):
            continue
        name = alloc.memorylocations[0].name
        if alloc.kind == "ExternalInput":
            if name != partition_name:
                in_names.append(name)
        elif alloc.kind == "ExternalOutput":
            out_names.append(name)
            out_avals.append(
                jax.core.ShapedArray(
                    tuple(alloc.tensor_shape), mybir.dt.np(alloc.dtype)
                )
            )
    bind_in_names = list(in_names)
    if partition_name is not None:
        bind_in_names.append(partition_name)

    def _body(*args):
        operands = list(args)
        if partition_name is not None:
            operands.append(bass2jax.partition_id_tensor())
        outs = bass2jax._bass_exec_p.bind(
            *operands,
            out_avals=tuple(out_avals),
            in_names=tuple(bind_in_names),
            out_names=tuple(out_names),
            lowering_input_output_aliases=(),
            sim_require_finite=True,
            sim_require_nnan=True,
            nc=nc,
        )
        return tuple(outs)

    devices = jax.devices()[:N_CORES]
    assert len(devices) == N_CORES
    mesh = Mesh(_np.asarray(devices), ("core",))
    fn = jax.jit(
        shard_map(
            _body,
            mesh=mesh,
            in_specs=(PartitionSpec("core"),) * len(in_names),
            out_specs=(PartitionSpec("core"),) * len(out_names),
            check_rep=False,
        )
    )
    return fn, out_names, mesh


def _get_runner():
    if "runner" not in _CACHE:
        _CACHE["runner"] = _make_runner()
    return _CACHE["runner"]


def _run_fallback(observed):
    """Slower but battle-tested path via run_bass_kernel_spmd."""
    from concourse.bass_utils import run_bass_kernel_spmd

    nc = _get_nc()
    in_maps = [
        {
            "observed": np.ascontiguousarray(
                observed[i * ROWS_PER_CORE : (i + 1) * ROWS_PER_CORE]
            )
        }
        for i in range(N_CORES)
    ]
    res = run_bass_kernel_spmd(nc, in_maps, list(range(N_CORES)))
    scale = np.concatenate(
        [res.results[i]["scale"] for i in range(N_CORES)], axis=0
    )
    zp = np.concatenate(
        [res.results[i]["zero_point"] for i in range(N_CORES)], axis=0
    )
    return scale, zp


def kernel(**inputs):
    observed = np.asarray(inputs["observed"], dtype=np.float32)
    assert observed.shape == (ROWS, COLS)
    try:
        fn, out_names, _ = _get_runner()
        outs = fn(observed)
        by_name = dict(zip(out_names, outs))
        return np.asarray(by_name["scale"]), np.asarray(by_name["zero_point"])
    except Exception:
        return _run_fallback(observed)
